# revision 11
# baseline (speedup 1.0000x reference)
"""Trainium2 Bass kernel for nn_COLoss_45457933860953.

Loss = mean over all pixels of weighted -log(conf gathered by instance)
     + mean over batches of (masked offset MSE sum / fg count).

Data-parallel over the batch dim: 16 batches -> 8 cores x 2 batches.
The instance map (values 0/1) is shipped as int8 (lossless) to cut DMA
bytes ~10%; C=2 turns the gather into a predicated copy; both loss
reductions use fused multiply+free-sum (scalar_tensor_tensor accum).

Each core emits [128, 6] per-partition partials:
  col 0: sum log(g)          (both batches)
  col 1: sum m*log(g)        (both batches)
  col 2: sum m*((g0-o0)^2 + (g1-o1)^2) batch 0
  col 3: same, batch 1
  col 4: count(m) batch 0
  col 5: count(m) batch 1
Host combines in float64:
  conf_loss = -(0.4*S1 + 0.6*S2)/N        (weight = 0.4 + 0.6*m)
  off_loss  = mean_b(sums_b / counts_b)
"""

import sys

if "/opt/trn_rl_repo" not in sys.path:
    sys.path.insert(0, "/opt/trn_rl_repo")

import numpy as np

import concourse.bass as bass
import concourse.tile as tile
from concourse import mybir
from concourse.bass_utils import run_bass_kernel_spmd

B, C, H, W = 16, 2, 512, 512
NCORES = 8
BPC = B // NCORES            # batches per core
P = 128                      # SBUF partitions
FREE = (H * W) // P          # 2048 free elems per partition per image
CHUNKS = (1536, 512)         # asymmetric: short last chunk = short tail
NCHUNK = len(CHUNKS)
NSETS = BPC * NCHUNK         # chunk-sets per core

F32 = mybir.dt.float32
I8 = mybir.dt.int8
AF = mybir.ActivationFunctionType
ALU = mybir.AluOpType


def _legalize_single_wait(nc):
    """This toolchain's walrus accepts at most ONE sync-wait on TPB compute
    instructions and rejects the EVENT_SEMAPHORE_RANGE_CLEAR InstISA that
    TileContext emits in its kernel tail. Drop the range clear (sems are
    not recycled in a one-shot NEFF) and hoist surplus waits onto
    standalone single-wait InstEventSemaphore carriers placed immediately
    before the instruction on the same engine queue (prefix waits on an
    in-order queue are semantically identical to instruction waits)."""
    cnt = 0
    for f in nc.m.functions:
        for blk in f.blocks:
            out = []
            for ins in blk.instructions:
                nm = type(ins).__name__
                if (nm == "InstISA" and
                        getattr(ins, "op_name", None) ==
                        "EVENT_SEMAPHORE_RANGE_CLEAR"):
                    continue
                si = getattr(ins, "sync_info", None)
                if si is not None and si.on_wait and len(si.on_wait) > 1:
                    waits = list(si.on_wait)
                    for w in waits[:-1]:
                        cnt += 1
                        out.append(mybir.InstEventSemaphore(
                            name=f"{ins.name}-hoist{cnt}",
                            engine=ins.engine,
                            ins=[], outs=[],
                            sync_info=mybir.SyncInfo(on_wait=[w],
                                                     on_update=[]),
                        ))
                    ins.sync_info = mybir.SyncInfo(
                        on_wait=[waits[-1]], on_update=list(si.on_update))
                out.append(ins)
            blk.instructions = out
    return nc


def build_nc(legalize=True):
    nc = bass.Bass("TRN2", target_bir_lowering=False, debug=False,
                   num_devices=NCORES)
    conf = nc.dram_tensor("conf", [BPC, C, H, W], F32, kind="ExternalInput")
    off = nc.dram_tensor("off", [BPC, C, H, W], F32, kind="ExternalInput")
    inst = nc.dram_tensor("inst", [BPC, 1, H, W], I8, kind="ExternalInput")
    gto = nc.dram_tensor("gto", [BPC, C, H, W], F32, kind="ExternalInput")
    out = nc.dram_tensor("partials", [P, 6], F32, kind="ExternalOutput")

    # [b, c, (p q), w] -> [p, b, c, (q w)]: partition p holds 4 contiguous
    # image rows; any column slice is contiguous per partition.
    conf_r = conf.rearrange("b c (p q) w -> p b c (q w)", p=P)
    off_r = off.rearrange("b c (p q) w -> p b c (q w)", p=P)
    gto_r = gto.rearrange("b c (p q) w -> p b c (q w)", p=P)
    inst_r = inst.rearrange("b c (p q) w -> p b (c q w)", p=P)

    def acc_tiles(pool, base, n):
        return [pool.tile([P, 1], F32, name=f"{base}{i}", tag=f"{base}{i}")
                for i in range(n)]

    with tile.TileContext(nc) as tc:
        with (
            tc.tile_pool(name="io", bufs=3) as io,
            tc.tile_pool(name="work", bufs=NSETS) as work,
            tc.tile_pool(name="acc", bufs=1) as accp,
        ):
            lg_s = acc_tiles(accp, "lg_s", NSETS)     # sum log(g) per set
            mlg_s = acc_tiles(accp, "mlg_s", NSETS)   # sum m*log(g) per set
            cnt_s = acc_tiles(accp, "cnt_s", NSETS)   # count(m) per set
            off_s = acc_tiles(accp, "off_s", NSETS * C)  # per set+channel
            zb = accp.tile([P, 1], F32)               # zero bias for ACT

            nc.vector.memset(zb[:], 0.0)

            for bi in range(BPC):
                col = 0
                for j, T in enumerate(CHUNKS):
                    si = bi * NCHUNK + j
                    cs = slice(col, col + T)
                    col += T

                    conf_t = io.tile([P, C, CHUNKS[0]], F32, name="conf_t",
                                     tag="conf_t")
                    nc.sync.dma_start(conf_t[:, :, :T], conf_r[:, bi, :, cs])
                    off_t = io.tile([P, C, CHUNKS[0]], F32, name="off_t",
                                    tag="off_t")
                    nc.sync.dma_start(off_t[:, :, :T], off_r[:, bi, :, cs])
                    gto_t = io.tile([P, C, CHUNKS[0]], F32, name="gto_t",
                                    tag="gto_t")
                    nc.sync.dma_start(gto_t[:, :, :T], gto_r[:, bi, :, cs])
                    mask_t = io.tile([P, CHUNKS[0]], I8, name="mask_t",
                                     tag="mask_t")
                    nc.sync.dma_start(mask_t[:, :T], inst_r[:, bi, cs])

                    mask = mask_t[:, :T]
                    g = conf_t[:, 0, :T]

                    # g = where(m, conf1, conf0), in place in channel 0;
                    # then g <- log(g) with free-sum into lg_s.
                    nc.vector.copy_predicated(g, mask, conf_t[:, 1, :T])
                    nc.scalar.activation(g, g, AF.Ln, bias=zb[:],
                                         accum_out=lg_s[si][:])
                    nc.vector.scalar_tensor_tensor(
                        out=g, in0=g, scalar=1.0, in1=mask,
                        op0=ALU.mult, op1=ALU.mult, accum_out=mlg_s[si][:])

                    # count: m*m = m, free-sum; in-place is value-preserving
                    nc.vector.scalar_tensor_tensor(
                        out=mask, in0=mask, scalar=1.0, in1=mask,
                        op0=ALU.mult, op1=ALU.mult, accum_out=cnt_s[si][:])

                    for c in range(C):
                        d = work.tile([P, CHUNKS[0]], F32, name=f"d{c}",
                                      tag=f"d{c}")
                        dv = d[:, :T]
                        nc.gpsimd.tensor_sub(dv, gto_t[:, c, :T],
                                             off_t[:, c, :T])
                        nc.scalar.activation(dv, dv, AF.Square, bias=zb[:])
                        nc.vector.scalar_tensor_tensor(
                            out=dv, in0=dv, scalar=1.0, in1=mask,
                            op0=ALU.mult, op1=ALU.mult,
                            accum_out=off_s[si * C + c][:])

            res = accp.tile([P, 6], F32)

            def tree_sum(dst, tiles):
                nc.vector.tensor_add(dst, tiles[0][:], tiles[1][:])
                for t in tiles[2:]:
                    nc.vector.tensor_add(dst, dst, t[:])

            tree_sum(res[:, 0:1], lg_s)
            tree_sum(res[:, 1:2], mlg_s)
            for bi in range(BPC):
                tree_sum(res[:, 2 + bi:3 + bi],
                         off_s[bi * NCHUNK * C:(bi + 1) * NCHUNK * C])
                tree_sum(res[:, 4 + bi:5 + bi],
                         cnt_s[bi * NCHUNK:(bi + 1) * NCHUNK])
            nc.sync.dma_start(out[:, :], res[:])

    return _legalize_single_wait(nc) if legalize else nc


_NC = None


def _get_nc():
    global _NC
    if _NC is None:
        _NC = build_nc()
    return _NC


def make_in_maps(confidence, offset, instance, gt_offset):
    confidence = np.ascontiguousarray(confidence, dtype=np.float32)
    offset = np.ascontiguousarray(offset, dtype=np.float32)
    gt_offset = np.ascontiguousarray(gt_offset, dtype=np.float32)
    inst8 = instance.astype(np.int8)     # values are 0/1: lossless
    in_maps = []
    for k in range(NCORES):
        sl = slice(BPC * k, BPC * (k + 1))
        in_maps.append({
            "conf": confidence[sl],
            "off": offset[sl],
            "inst": inst8[sl],
            "gto": gt_offset[sl],
        })
    return in_maps


def combine_partials(parts):
    """parts: list of 8 arrays [128, 6] -> scalar loss (float64)."""
    s1 = sum(p[:, 0].sum(dtype=np.float64) for p in parts)
    s2 = sum(p[:, 1].sum(dtype=np.float64) for p in parts)
    n = float(B * H * W)
    conf_loss = -(0.4 * s1 + 0.6 * s2) / n
    off_loss = 0.0
    for p in parts:
        for bi in range(BPC):
            s = p[:, 2 + bi].sum(dtype=np.float64)
            cnt = p[:, 4 + bi].sum(dtype=np.float64)
            if cnt > 0.5:
                off_loss += s / cnt
    off_loss /= B
    return conf_loss + off_loss


def kernel(confidence, offset, instance, gt_offset):
    nc = _get_nc()
    in_maps = make_in_maps(confidence, offset, instance, gt_offset)
    res = run_bass_kernel_spmd(nc, in_maps, core_ids=list(range(NCORES)))
    parts = [r["partials"] for r in res.results]
    return np.array(combine_partials(parts), dtype=np.float32)


# revision 14
# speedup vs baseline: 1.1467x; 1.1467x over previous
"""Trainium2 Bass kernel for nn_COLoss_45457933860953.

Loss = mean over all pixels of weighted -log(conf gathered by instance)
     + mean over batches of (masked offset MSE sum / fg count).

Data-parallel over the batch dim: 16 batches -> 8 cores x 2 batches.
The instance map (values 0/1) is shipped as int8 (lossless) to cut DMA
bytes ~10%; C=2 turns the gather into a predicated copy; both loss
reductions use fused multiply+free-sum (scalar_tensor_tensor accum).

Each core emits [128, 6] per-partition partials:
  col 0: sum log(g)          (both batches)
  col 1: sum m*log(g)        (both batches)
  col 2: sum m*((g0-o0)^2 + (g1-o1)^2) batch 0
  col 3: same, batch 1
  col 4: count(m) batch 0
  col 5: count(m) batch 1
Host combines in float64:
  conf_loss = -(0.4*S1 + 0.6*S2)/N        (weight = 0.4 + 0.6*m)
  off_loss  = mean_b(sums_b / counts_b)
"""

import sys

if "/opt/trn_rl_repo" not in sys.path:
    sys.path.insert(0, "/opt/trn_rl_repo")

import numpy as np

import concourse.bass as bass
import concourse.tile as tile
from concourse import mybir
from concourse.bass_utils import run_bass_kernel_spmd

B, C, H, W = 16, 2, 512, 512
NCORES = 8
BPC = B // NCORES            # batches per core
P = 128                      # SBUF partitions
FREE = (H * W) // P          # 2048 free elems per partition per image
CHUNKS = (768, 768, 512)     # asymmetric: short last chunk = short tail
NCHUNK = len(CHUNKS)
NSETS = BPC * NCHUNK         # chunk-sets per core

F32 = mybir.dt.float32
I8 = mybir.dt.int8
AF = mybir.ActivationFunctionType
ALU = mybir.AluOpType


def _legalize_single_wait(nc):
    """This toolchain's walrus accepts at most ONE sync-wait on TPB compute
    instructions and rejects the EVENT_SEMAPHORE_RANGE_CLEAR InstISA that
    TileContext emits in its kernel tail. Drop the range clear (sems are
    not recycled in a one-shot NEFF) and hoist surplus waits onto
    standalone single-wait InstEventSemaphore carriers placed immediately
    before the instruction on the same engine queue (prefix waits on an
    in-order queue are semantically identical to instruction waits)."""
    cnt = 0
    for f in nc.m.functions:
        for blk in f.blocks:
            out = []
            for ins in blk.instructions:
                nm = type(ins).__name__
                if (nm == "InstISA" and
                        getattr(ins, "op_name", None) ==
                        "EVENT_SEMAPHORE_RANGE_CLEAR"):
                    continue
                si = getattr(ins, "sync_info", None)
                if si is not None and si.on_wait and len(si.on_wait) > 1:
                    waits = list(si.on_wait)
                    for w in waits[:-1]:
                        cnt += 1
                        out.append(mybir.InstEventSemaphore(
                            name=f"{ins.name}-hoist{cnt}",
                            engine=ins.engine,
                            ins=[], outs=[],
                            sync_info=mybir.SyncInfo(on_wait=[w],
                                                     on_update=[]),
                        ))
                    ins.sync_info = mybir.SyncInfo(
                        on_wait=[waits[-1]], on_update=list(si.on_update))
                out.append(ins)
            blk.instructions = out
    return nc


def build_nc(legalize=True):
    nc = bass.Bass("TRN2", target_bir_lowering=False, debug=False,
                   num_devices=NCORES)
    conf = nc.dram_tensor("conf", [BPC, C, H, W], F32, kind="ExternalInput")
    off = nc.dram_tensor("off", [BPC, C, H, W], F32, kind="ExternalInput")
    inst = nc.dram_tensor("inst", [BPC, 1, H, W], I8, kind="ExternalInput")
    gto = nc.dram_tensor("gto", [BPC, C, H, W], F32, kind="ExternalInput")
    out = nc.dram_tensor("partials", [P, 6], F32, kind="ExternalOutput")

    # [b, c, (p q), w] -> [p, b, c, (q w)]: partition p holds 4 contiguous
    # image rows; any column slice is contiguous per partition.
    conf_r = conf.rearrange("b c (p q) w -> p b c (q w)", p=P)
    off_r = off.rearrange("b c (p q) w -> p b c (q w)", p=P)
    gto_r = gto.rearrange("b c (p q) w -> p b c (q w)", p=P)
    inst_r = inst.rearrange("b c (p q) w -> p b (c q w)", p=P)

    def acc_tiles(pool, base, n):
        return [pool.tile([P, 1], F32, name=f"{base}{i}", tag=f"{base}{i}")
                for i in range(n)]

    with tile.TileContext(nc) as tc:
        with (
            tc.tile_pool(name="io", bufs=4) as io,
            tc.tile_pool(name="work", bufs=4) as work,
            tc.tile_pool(name="acc", bufs=1) as accp,
        ):
            lg_s = acc_tiles(accp, "lg_s", NSETS)     # sum log(g) per set
            mlg_s = acc_tiles(accp, "mlg_s", NSETS)   # sum m*log(g) per set
            cnt_s = acc_tiles(accp, "cnt_s", BPC)     # count(m) per batch
            off_s = acc_tiles(accp, "off_s", NSETS * C)  # per set+channel
            zb = accp.tile([P, 1], F32)               # zero bias for ACT

            nc.vector.memset(zb[:], 0.0)

            for bi in range(BPC):
                # full-batch mask: one DMA, count once on ACT (off the
                # critical path), chunk slices feed the masked reductions
                mask_t = io.tile([P, FREE], I8, name="mask_t", tag="mask_t",
                                 bufs=2)
                nc.sync.dma_start(mask_t[:], inst_r[:, bi, :])
                instf = work.tile([P, FREE], F32, name="instf", tag="instf",
                                  bufs=2)
                nc.scalar.activation(instf[:], mask_t[:], AF.Copy,
                                     accum_out=cnt_s[bi][:])

                col = 0
                for j, T in enumerate(CHUNKS):
                    si = bi * NCHUNK + j
                    cs = slice(col, col + T)

                    conf_t = io.tile([P, C, CHUNKS[0]], F32, name="conf_t",
                                     tag="conf_t")
                    nc.sync.dma_start(conf_t[:, :, :T], conf_r[:, bi, :, cs])
                    off_t = io.tile([P, C, CHUNKS[0]], F32, name="off_t",
                                    tag="off_t")
                    nc.sync.dma_start(off_t[:, :, :T], off_r[:, bi, :, cs])
                    gto_t = io.tile([P, C, CHUNKS[0]], F32, name="gto_t",
                                    tag="gto_t")
                    nc.sync.dma_start(gto_t[:, :, :T], gto_r[:, bi, :, cs])

                    mask = mask_t[:, cs]
                    col += T
                    g = conf_t[:, 0, :T]

                    # g = where(m, conf1, conf0), in place in channel 0;
                    # then g <- log(g) with free-sum into lg_s.
                    nc.vector.copy_predicated(g, mask, conf_t[:, 1, :T])
                    nc.scalar.activation(g, g, AF.Ln, bias=zb[:],
                                         accum_out=lg_s[si][:])
                    nc.vector.scalar_tensor_tensor(
                        out=g, in0=g, scalar=1.0, in1=mask,
                        op0=ALU.mult, op1=ALU.mult, accum_out=mlg_s[si][:])

                    for c in range(C):
                        d = work.tile([P, CHUNKS[0]], F32, name=f"d{c}",
                                      tag=f"d{c}")
                        dv = d[:, :T]
                        # split the subtractions across Pool and DVE
                        eng = nc.gpsimd if c == 0 else nc.vector
                        eng.tensor_sub(dv, gto_t[:, c, :T], off_t[:, c, :T])
                        nc.scalar.activation(dv, dv, AF.Square, bias=zb[:])
                        nc.vector.scalar_tensor_tensor(
                            out=dv, in0=dv, scalar=1.0, in1=mask,
                            op0=ALU.mult, op1=ALU.mult,
                            accum_out=off_s[si * C + c][:])

            res = accp.tile([P, 6], F32)

            def tree_sum(dst, tiles):
                nc.vector.tensor_add(dst, tiles[0][:], tiles[1][:])
                for t in tiles[2:]:
                    nc.vector.tensor_add(dst, dst, t[:])

            tree_sum(res[:, 0:1], lg_s)
            tree_sum(res[:, 1:2], mlg_s)
            for bi in range(BPC):
                tree_sum(res[:, 2 + bi:3 + bi],
                         off_s[bi * NCHUNK * C:(bi + 1) * NCHUNK * C])
                nc.vector.tensor_copy(res[:, 4 + bi:5 + bi], cnt_s[bi][:])
            nc.sync.dma_start(out[:, :], res[:])

    return _legalize_single_wait(nc) if legalize else nc


_NC = None


def _get_nc():
    global _NC
    if _NC is None:
        _NC = build_nc()
    return _NC


def make_in_maps(confidence, offset, instance, gt_offset):
    confidence = np.ascontiguousarray(confidence, dtype=np.float32)
    offset = np.ascontiguousarray(offset, dtype=np.float32)
    gt_offset = np.ascontiguousarray(gt_offset, dtype=np.float32)
    inst8 = instance.astype(np.int8)     # values are 0/1: lossless
    in_maps = []
    for k in range(NCORES):
        sl = slice(BPC * k, BPC * (k + 1))
        in_maps.append({
            "conf": confidence[sl],
            "off": offset[sl],
            "inst": inst8[sl],
            "gto": gt_offset[sl],
        })
    return in_maps


def combine_partials(parts):
    """parts: list of 8 arrays [128, 6] -> scalar loss (float64)."""
    s1 = sum(p[:, 0].sum(dtype=np.float64) for p in parts)
    s2 = sum(p[:, 1].sum(dtype=np.float64) for p in parts)
    n = float(B * H * W)
    conf_loss = -(0.4 * s1 + 0.6 * s2) / n
    off_loss = 0.0
    for p in parts:
        for bi in range(BPC):
            s = p[:, 2 + bi].sum(dtype=np.float64)
            cnt = p[:, 4 + bi].sum(dtype=np.float64)
            if cnt > 0.5:
                off_loss += s / cnt
    off_loss /= B
    return conf_loss + off_loss


def kernel(confidence, offset, instance, gt_offset):
    nc = _get_nc()
    in_maps = make_in_maps(confidence, offset, instance, gt_offset)
    res = run_bass_kernel_spmd(nc, in_maps, core_ids=list(range(NCORES)))
    parts = [r["partials"] for r in res.results]
    return np.array(combine_partials(parts), dtype=np.float32)


# revision 18
# speedup vs baseline: 1.2162x; 1.0606x over previous
"""Trainium2 Bass kernel for nn_COLoss_45457933860953.

Loss = mean over all pixels of weighted -log(conf gathered by instance)
     + mean over batches of (masked offset MSE sum / fg count).

Data-parallel over the batch dim: 16 batches -> 8 cores x 2 batches.
The instance map (values 0/1) is shipped as int8 (lossless) to cut DMA
bytes ~10%; C=2 turns the gather into a predicated copy; both loss
reductions use fused multiply+free-sum (scalar_tensor_tensor accum).

Each core emits [128, 6] per-partition partials:
  col 0: sum log(g)          (both batches)
  col 1: sum m*log(g)        (both batches)
  col 2: sum m*((g0-o0)^2 + (g1-o1)^2) batch 0
  col 3: same, batch 1
  col 4: count(m) batch 0
  col 5: count(m) batch 1
Host combines in float64:
  conf_loss = -(0.4*S1 + 0.6*S2)/N        (weight = 0.4 + 0.6*m)
  off_loss  = mean_b(sums_b / counts_b)
"""

import sys

if "/opt/trn_rl_repo" not in sys.path:
    sys.path.insert(0, "/opt/trn_rl_repo")

import numpy as np

import concourse.bass as bass
import concourse.tile as tile
from concourse import mybir
from concourse.bass_utils import run_bass_kernel_spmd

B, C, H, W = 16, 2, 512, 512
NCORES = 8
BPC = B // NCORES            # batches per core
P = 128                      # SBUF partitions
FREE = (H * W) // P          # 2048 free elems per partition per image
CHUNKS = (1024, 1024)        # 4KB per partition per DMA = full DMA BW
NCHUNK = len(CHUNKS)
NSETS = BPC * NCHUNK         # chunk-sets per core

F32 = mybir.dt.float32
I8 = mybir.dt.int8
AF = mybir.ActivationFunctionType
ALU = mybir.AluOpType


def _legalize_single_wait(nc):
    """This toolchain's walrus accepts at most ONE sync-wait on TPB compute
    instructions and rejects the EVENT_SEMAPHORE_RANGE_CLEAR InstISA that
    TileContext emits in its kernel tail. Drop the range clear (sems are
    not recycled in a one-shot NEFF) and hoist surplus waits onto
    standalone single-wait InstEventSemaphore carriers placed immediately
    before the instruction on the same engine queue (prefix waits on an
    in-order queue are semantically identical to instruction waits)."""
    cnt = 0
    for f in nc.m.functions:
        for blk in f.blocks:
            out = []
            for ins in blk.instructions:
                nm = type(ins).__name__
                if (nm == "InstISA" and
                        getattr(ins, "op_name", None) ==
                        "EVENT_SEMAPHORE_RANGE_CLEAR"):
                    continue
                si = getattr(ins, "sync_info", None)
                if si is not None and si.on_wait and len(si.on_wait) > 1:
                    waits = list(si.on_wait)
                    for w in waits[:-1]:
                        cnt += 1
                        out.append(mybir.InstEventSemaphore(
                            name=f"{ins.name}-hoist{cnt}",
                            engine=ins.engine,
                            ins=[], outs=[],
                            sync_info=mybir.SyncInfo(on_wait=[w],
                                                     on_update=[]),
                        ))
                    ins.sync_info = mybir.SyncInfo(
                        on_wait=[waits[-1]], on_update=list(si.on_update))
                out.append(ins)
            blk.instructions = out
    return nc


def build_nc(legalize=True):
    nc = bass.Bass("TRN2", target_bir_lowering=False, debug=False,
                   num_devices=NCORES)
    conf = nc.dram_tensor("conf", [BPC, C, H, W], F32, kind="ExternalInput")
    off = nc.dram_tensor("off", [BPC, C, H, W], F32, kind="ExternalInput")
    inst = nc.dram_tensor("inst", [BPC, 1, H, W], I8, kind="ExternalInput")
    gto = nc.dram_tensor("gto", [BPC, C, H, W], F32, kind="ExternalInput")
    out = nc.dram_tensor("partials", [P, 6], F32, kind="ExternalOutput")

    # [b, c, (p q), w] -> [p, b, c, (q w)]: partition p holds 4 contiguous
    # image rows; any column slice is contiguous per partition.
    conf_r = conf.rearrange("b c (p q) w -> p b c (q w)", p=P)
    off_r = off.rearrange("b c (p q) w -> p b c (q w)", p=P)
    gto_r = gto.rearrange("b c (p q) w -> p b c (q w)", p=P)
    inst_r = inst.rearrange("b c (p q) w -> p b (c q w)", p=P)

    def acc_tiles(pool, base, n):
        return [pool.tile([P, 1], F32, name=f"{base}{i}", tag=f"{base}{i}")
                for i in range(n)]

    with tile.TileContext(nc) as tc:
        with (
            tc.tile_pool(name="io", bufs=4) as io,
            tc.tile_pool(name="work", bufs=4) as work,
            tc.tile_pool(name="acc", bufs=1) as accp,
        ):
            # one extra slot: the last chunk's conf path runs in 2 halves
            lg_s = acc_tiles(accp, "lg_s", NSETS + 1)   # sum log(g)
            mlg_s = acc_tiles(accp, "mlg_s", NSETS + 1)  # sum m*log(g)
            cnt_s = acc_tiles(accp, "cnt_s", BPC)     # count(m) per batch
            off_s = acc_tiles(accp, "off_s", NSETS * C)  # per set+channel
            zb = accp.tile([P, 1], F32)               # zero bias for ACT

            nc.vector.memset(zb[:], 0.0)

            for bi in range(BPC):
                # full-batch mask: one DMA, count once on ACT (off the
                # critical path), chunk slices feed the masked reductions
                mask_t = io.tile([P, FREE], I8, name="mask_t", tag="mask_t",
                                 bufs=2)
                nc.sync.dma_start(mask_t[:], inst_r[:, bi, :])
                instf = work.tile([P, FREE], F32, name="instf", tag="instf",
                                  bufs=2)
                nc.scalar.activation(instf[:], mask_t[:], AF.Copy,
                                     accum_out=cnt_s[bi][:])

                col = 0
                for j, T in enumerate(CHUNKS):
                    si = bi * NCHUNK + j
                    cs = slice(col, col + T)
                    last = (bi == BPC - 1 and j == NCHUNK - 1)

                    conf_t = io.tile([P, C, CHUNKS[0]], F32, name="conf_t",
                                     tag="conf_t")
                    off_t = io.tile([P, C, CHUNKS[0]], F32, name="off_t",
                                    tag="off_t")
                    gto_t = io.tile([P, C, CHUNKS[0]], F32, name="gto_t",
                                    tag="gto_t")
                    if last:
                        # offset data first: its chains finish while the
                        # conf bytes are still streaming
                        nc.sync.dma_start(off_t[:, :, :T],
                                          off_r[:, bi, :, cs])
                        nc.sync.dma_start(gto_t[:, :, :T],
                                          gto_r[:, bi, :, cs])
                        nc.sync.dma_start(conf_t[:, :, :T],
                                          conf_r[:, bi, :, cs])
                    else:
                        nc.sync.dma_start(conf_t[:, :, :T],
                                          conf_r[:, bi, :, cs])
                        nc.sync.dma_start(off_t[:, :, :T],
                                          off_r[:, bi, :, cs])
                        nc.sync.dma_start(gto_t[:, :, :T],
                                          gto_r[:, bi, :, cs])

                    mask = mask_t[:, cs]
                    col += T

                    for c in range(C):
                        d = work.tile([P, CHUNKS[0]], F32, name=f"d{c}",
                                      tag=f"d{c}")
                        dv = d[:, :T]
                        nc.vector.tensor_sub(dv, gto_t[:, c, :T],
                                             off_t[:, c, :T])
                        nc.scalar.activation(dv, dv, AF.Square, bias=zb[:])
                        nc.vector.scalar_tensor_tensor(
                            out=dv, in0=dv, scalar=1.0, in1=mask,
                            op0=ALU.mult, op1=ALU.mult,
                            accum_out=off_s[si * C + c][:])

                    # g = where(m, conf1, conf0), in place in channel 0;
                    # then g <- log(g) with free-sum. The very last chunk
                    # runs in halves so the post-stream serial chain is
                    # half as long (second half accumulates to slot NSETS).
                    halves = 2 if last else 1
                    hs = T // halves
                    for h in range(halves):
                        acc_i = si if h == 0 else NSETS
                        hsl = slice(h * hs, (h + 1) * hs)
                        g = conf_t[:, 0, hsl]
                        mh = mask_t[:, col - T + h * hs:col - T + (h + 1) * hs]
                        nc.vector.copy_predicated(g, mh, conf_t[:, 1, hsl])
                        nc.scalar.activation(g, g, AF.Ln, bias=zb[:],
                                             accum_out=lg_s[acc_i][:])
                        nc.vector.scalar_tensor_tensor(
                            out=g, in0=g, scalar=1.0, in1=mh,
                            op0=ALU.mult, op1=ALU.mult,
                            accum_out=mlg_s[acc_i][:])

            res = accp.tile([P, 6], F32)

            def tree_sum(dst, tiles):
                nc.vector.tensor_add(dst, tiles[0][:], tiles[1][:])
                for t in tiles[2:]:
                    nc.vector.tensor_add(dst, dst, t[:])

            tree_sum(res[:, 0:1], lg_s)
            tree_sum(res[:, 1:2], mlg_s)
            for bi in range(BPC):
                tree_sum(res[:, 2 + bi:3 + bi],
                         off_s[bi * NCHUNK * C:(bi + 1) * NCHUNK * C])
                nc.vector.tensor_copy(res[:, 4 + bi:5 + bi], cnt_s[bi][:])
            nc.sync.dma_start(out[:, :], res[:])

    return _legalize_single_wait(nc) if legalize else nc


_NC = None


def _get_nc():
    global _NC
    if _NC is None:
        _NC = build_nc()
    return _NC


def make_in_maps(confidence, offset, instance, gt_offset):
    confidence = np.ascontiguousarray(confidence, dtype=np.float32)
    offset = np.ascontiguousarray(offset, dtype=np.float32)
    gt_offset = np.ascontiguousarray(gt_offset, dtype=np.float32)
    inst8 = instance.astype(np.int8)     # values are 0/1: lossless
    in_maps = []
    for k in range(NCORES):
        sl = slice(BPC * k, BPC * (k + 1))
        in_maps.append({
            "conf": confidence[sl],
            "off": offset[sl],
            "inst": inst8[sl],
            "gto": gt_offset[sl],
        })
    return in_maps


def combine_partials(parts):
    """parts: list of 8 arrays [128, 6] -> scalar loss (float64)."""
    s1 = sum(p[:, 0].sum(dtype=np.float64) for p in parts)
    s2 = sum(p[:, 1].sum(dtype=np.float64) for p in parts)
    n = float(B * H * W)
    conf_loss = -(0.4 * s1 + 0.6 * s2) / n
    off_loss = 0.0
    for p in parts:
        for bi in range(BPC):
            s = p[:, 2 + bi].sum(dtype=np.float64)
            cnt = p[:, 4 + bi].sum(dtype=np.float64)
            if cnt > 0.5:
                off_loss += s / cnt
    off_loss /= B
    return conf_loss + off_loss


def kernel(confidence, offset, instance, gt_offset):
    nc = _get_nc()
    in_maps = make_in_maps(confidence, offset, instance, gt_offset)
    res = run_bass_kernel_spmd(nc, in_maps, core_ids=list(range(NCORES)))
    parts = [r["partials"] for r in res.results]
    return np.array(combine_partials(parts), dtype=np.float32)
